# revision 28
# baseline (speedup 1.0000x reference)
"""GPT block (LN -> causal MHA -> residual -> LN -> MLP -> residual) on 8 trn2 cores.

Sharding: core c = (batch b = c//2, parity o = c%2). Each core owns the
interleaved tokens o::2 of its batch. The host PERMUTES tokens so the core's
own tokens come first: xb = concat(x[o::2], x[1-o::2]). Causality in permuted
space becomes two triangles (key-half 0: k <= i; key-half 1: k < i or k <= i
depending on parity), handled by two multiplicative {0,1} masks supplied per
core. Q's transposed LN data is just the first TQ columns of XT, so layernorm1
runs once over the 2048 permuted tokens. K/V are computed redundantly by the
two cores of a batch. The MLP is token-parallel. No cross-core communication.

All matmuls run in bf16 (fp32 PSUM accumulation); layernorm/softmax
normalization stats stay fp32. LN outputs are converted to bf16 before their
PE transposes (1 cycle/row vs 2 for f32) and the PSUM->SBUF copies are split
between ACT and DVE. LN's rstd is computed as exp(-0.5*ln(var+eps)) so every
ACT function stays in one activation table (no ACT_TABLE_LOAD churn). Phase
order is software-pipelined: LN1 interleaves with the first projection group,
attention is split into query groups 0..511 / 512..1023, and the second
group's heads interleave with the first group's MLP so exp (ACT) hides under
matmul (PE) work.
"""

import sys

if "/opt/trn_rl_repo" not in sys.path:
    sys.path.insert(0, "/opt/trn_rl_repo")

import numpy as np
import ml_dtypes

import concourse.bass as bass
import concourse.tile as tile
from concourse import mybir
from concourse.bass_utils import run_bass_kernel_spmd
from concourse.masks import make_identity

B, T, D, H, HD = 4, 2048, 1024, 16, 64
FF = 4 * D
P = 128
NB = T // P        # 16 permuted key blocks (8 per half)
TQ = T // 2        # 1024 query tokens per core
NQ = TQ // P       # 8 query blocks per core
NC_DCH = D // P    # 8 contraction chunks over D
EPS = 1e-5
F32 = mybir.dt.float32
BF16 = mybir.dt.bfloat16
PT_TOTAL = 128 * 8 * 9  # 9216 packed score columns per head


def _pt_off(J, half):
    """Offset of the (J, half) block in the packed pt row. slen = 128*(8-J)."""
    return 128 * J * (17 - J) + half * 128 * (8 - J)


def _ln_chain(nc, lnp, src_ap, eps_sb, gtile, btile, use_act=True):
    """LN over rows of src_ap [128, D] (f32) -> bf16 tile [128, D].
    rstd = exp(-0.5*ln(var+eps)) keeps every ACT func in the
    natural_log_exp_and_others table (no ACT_TABLE_LOAD churn)."""
    stats = lnp.tile([P, 2, 6], F32, tag="stats")
    for s in range(2):
        nc.vector.bn_stats(out=stats[:, s, :], in_=src_ap[:, s * 512:(s + 1) * 512])
    mv = lnp.tile([P, 2], F32, tag="mv")
    nc.vector.bn_aggr(out=mv, in_=stats)
    lv = lnp.tile([P, 1], F32, tag="lv")
    nc.scalar.activation(out=lv, in_=mv[:, 1:2],
                         func=mybir.ActivationFunctionType.Ln,
                         bias=eps_sb, scale=1.0)
    rstd = lnp.tile([P, 1], F32, tag="rstd")
    nc.scalar.activation(out=rstd, in_=lv,
                         func=mybir.ActivationFunctionType.Exp, scale=-0.5)
    xn = lnp.tile([P, D], F32, tag="xn")
    nc.vector.tensor_scalar(out=xn, in0=src_ap, scalar1=mv[:, 0:1], scalar2=rstd,
                            op0=mybir.AluOpType.subtract, op1=mybir.AluOpType.mult)
    if gtile is not None:
        nc.vector.tensor_mul(xn, xn, gtile)
    if btile is not None:
        nc.vector.tensor_add(xn, xn, btile)
    xnb = lnp.tile([P, D], BF16, tag="xnb")
    if use_act:
        nc.scalar.activation(out=xnb, in_=xn,
                             func=mybir.ActivationFunctionType.Copy, scale=1.0)
    else:
        nc.vector.tensor_copy(xnb, xn)
    return xnb


def _tr_to_T(nc, pst, xnb, dst, dst_col, id_bf, use_act=True):
    """Transpose bf16 [128, D] into dst[:, c, dst_col:dst_col+128] per D-chunk.
    PE transposes; PSUM->SBUF copies alternate between ACT and DVE (or DVE
    only when ACT is hot)."""
    for c in range(NC_DCH):
        tp = pst.tile([P, P], BF16, tag="tp")
        nc.tensor.transpose(tp, xnb[:, c * P:(c + 1) * P], id_bf)
        if use_act and c % 2 == 0:
            nc.scalar.activation(out=dst[:, c, dst_col:dst_col + P], in_=tp,
                                 func=mybir.ActivationFunctionType.Copy, scale=1.0)
        else:
            nc.vector.tensor_copy(dst[:, c, dst_col:dst_col + P], tp)


def build_program(apply_g1=False, apply_g2=False):
    nc = bass.Bass()
    xb = nc.declare_dram_parameter("xb", [T, D], F32, isOutput=False)
    wq = nc.declare_dram_parameter("wq", [8, P, NC_DCH, P], BF16, isOutput=False)
    wk = nc.declare_dram_parameter("wk", [8, P, NC_DCH, P], BF16, isOutput=False)
    wv = nc.declare_dram_parameter("wv", [8, P, NC_DCH, P], BF16, isOutput=False)
    w1t = nc.declare_dram_parameter("w1t", [32, P, NC_DCH, P], BF16, isOutput=False)
    w2t = nc.declare_dram_parameter("w2t", [8, P, 32, P], BF16, isOutput=False)
    b1t = nc.declare_dram_parameter("b1t", [P, 32], F32, isOutput=False)
    b2t = nc.declare_dram_parameter("b2t", [P, 8], F32, isOutput=False)
    maskA = nc.declare_dram_parameter("maskA", [P, P], BF16, isOutput=False)
    maskB = nc.declare_dram_parameter("maskB", [P, P], BF16, isOutput=False)
    gb = {}
    if apply_g1:
        gb["g1"] = nc.declare_dram_parameter("g1v", [D], F32, isOutput=False)
        gb["be1"] = nc.declare_dram_parameter("be1v", [D], F32, isOutput=False)
    if apply_g2:
        gb["g2"] = nc.declare_dram_parameter("g2v", [D], F32, isOutput=False)
        gb["be2"] = nc.declare_dram_parameter("be2v", [D], F32, isOutput=False)
    out_d = nc.declare_dram_parameter("out", [TQ, D], F32, isOutput=True)

    Exp = mybir.ActivationFunctionType.Exp
    Relu = mybir.ActivationFunctionType.Relu

    with tile.TileContext(nc) as tc:
        with tc.tile_pool(name="consts", bufs=1) as consts, \
             tc.tile_pool(name="big", bufs=1) as big:
            id_f32 = consts.tile([P, P], F32)
            make_identity(nc, id_f32)
            id_bf = consts.tile([P, P], BF16)
            make_identity(nc, id_bf)
            eps_sb = consts.tile([P, 1], F32)
            nc.vector.memset(eps_sb, EPS)
            mA = consts.tile([P, P], BF16)
            nc.sync.dma_start(out=mA, in_=maskA[:, :])
            mB = consts.tile([P, P], BF16)
            nc.sync.dma_start(out=mB, in_=maskB[:, :])
            b1_sb = consts.tile([P, 32], F32)
            nc.sync.dma_start(out=b1_sb, in_=b1t[:, :])
            b2_sb = consts.tile([P, 8], F32)
            nc.sync.dma_start(out=b2_sb, in_=b2t[:, :])

            def bcast(name):
                t = consts.tile([P, D], F32, tag=f"bc_{name}")
                src = gb[name]
                ap = bass.AP(tensor=src.tensor if hasattr(src, "tensor") else src[:].tensor,
                             offset=src[:].offset, ap=[[0, P]] + list(src[:].ap))
                nc.sync.dma_start(out=t, in_=ap)
                return t

            g1_t = bcast("g1") if apply_g1 else None
            be1_t = bcast("be1") if apply_g1 else None
            g2_t = bcast("g2") if apply_g2 else None
            be2_t = bcast("be2") if apply_g2 else None

            KT = big.tile([P, 8, T], BF16)         # per pair: K^T over all keys
            VA = big.tile([P, 256, 65], BF16)      # V^T blocks + ones col,
            #                                        flat f = (h*2+half)*8+J
            QT = big.tile([P, 8, TQ], BF16)        # per pair: Q^T, my tokens
            xv = big.tile([P, NQ, D], F32)         # residual stream, my tokens

            nc.vector.memset(VA[:, :, 64:65], 1.0)

            # packed pt offsets for the two query groups
            def off0(J, half):  # g0: queries J*128..511, width (4-J)*128
                return sum(2 * (4 - Jp) * P for Jp in range(J)) + half * (4 - J) * P

            def off1(J, half):  # g1: queries 512..1023, width 512
                return (2 * J + half) * 512

            def epilogue(cscr, trp, ot, h, kb):
                ot_sb = cscr.tile([65, P], F32, tag="otsb")
                nc.vector.tensor_copy(ot_sb, ot)
                o_ps = trp.tile([P, 65], F32, tag="tf32")
                nc.tensor.transpose(o_ps, ot_sb, id_f32[0:65, 0:65])
                rd = cscr.tile([P, 1], F32, tag="rd")
                nc.vector.reciprocal(rd, o_ps[:, 64:65])
                osc = cscr.tile([P, 64], F32, tag="osc")
                nc.vector.tensor_scalar_mul(osc, o_ps[:, 0:64], rd)
                nc.vector.tensor_add(xv[:, kb, h * 64:(h + 1) * 64],
                                     xv[:, kb, h * 64:(h + 1) * 64], osc)

            # ---- Phases A+B+C0 share a scope: XT lives only here ----
            with tc.tile_pool(name="xtp", bufs=1) as xtp, \
                 tc.tile_pool(name="lnp", bufs=3) as lnp, \
                 tc.tile_pool(name="lnsrc", bufs=3) as lnsrc, \
                 tc.tile_pool(name="wp", bufs=2) as wp, \
                 tc.tile_pool(name="scr", bufs=4) as scr, \
                 tc.tile_pool(name="pt0p", bufs=2) as pt0p, \
                 tc.tile_pool(name="cscr0", bufs=3) as cscr0, \
                 tc.tile_pool(name="mm512", bufs=3, space="PSUM") as mm512:
                XT = xtp.tile([P, NC_DCH, T], BF16)  # LN1(xb)^T, permuted tokens

                def proj(wt_pool_tag, wdram, pr, cols, dst_ap):
                    w_p = wp.tile([P, NC_DCH, P], BF16, tag=wt_pool_tag)
                    nc.scalar.dma_start(out=w_p, in_=wdram[pr])
                    ps = mm512.tile([P, 512], F32, tag="mm")
                    for c in range(NC_DCH):
                        nc.tensor.matmul(ps, lhsT=w_p[:, c, :], rhs=XT[:, c, cols],
                                         start=(c == 0), stop=(c == NC_DCH - 1))
                    return ps

                def v_proj(tr65, pr, tg, cols, w_p=None):
                    if w_p is None:
                        w_p = wp.tile([P, NC_DCH, P], BF16, tag="wv")
                        nc.scalar.dma_start(out=w_p, in_=wv[pr])
                    ps = mm512.tile([P, 512], F32, tag="mm")
                    for c in range(NC_DCH):
                        nc.tensor.matmul(ps, lhsT=w_p[:, c, :], rhs=XT[:, c, cols],
                                         start=(c == 0), stop=(c == NC_DCH - 1))
                    vt_sb = scr.tile([P, 512], BF16, tag="vt")
                    nc.vector.tensor_copy(vt_sb, ps)
                    for hh in range(2):
                        h = pr * 2 + hh
                        for s in range(4):
                            blk = tg * 4 + s
                            f = (h * 2 + blk // 8) * 8 + blk % 8
                            tps = tr65.tile([P, 65], BF16, tag="tr")
                            nc.tensor.transpose(
                                tps[:, 0:64],
                                vt_sb[hh * 64:(hh + 1) * 64, s * P:(s + 1) * P],
                                id_bf[hh * 64:(hh + 1) * 64, hh * 64:hh * 64 + 64])
                            nc.vector.tensor_copy(VA[:, f, 0:64], tps[:, 0:64])
                    return w_p

                with tc.tile_pool(name="pst", bufs=2, space="PSUM") as pst, \
                     tc.tile_pool(name="tr65", bufs=2, space="PSUM") as tr65:
                    def a_block(blk):
                        x_t = lnsrc.tile([P, D], F32, tag="xsrc")
                        nc.sync.dma_start(out=x_t, in_=xb[blk * P:(blk + 1) * P, :])
                        xnb = _ln_chain(nc, lnp, x_t, eps_sb, g1_t, be1_t)
                        _tr_to_T(nc, pst, xnb, XT, blk * P, id_bf)

                    def b_tg0(pr):
                        cols = slice(0, 512)
                        nc.vector.tensor_copy(KT[:, pr, cols],
                                              proj("wk", wk, pr, cols, None))
                        v_proj(tr65, pr, 0, cols)
                        nc.vector.tensor_copy(QT[:, pr, cols],
                                              proj("wq", wq, pr, cols, None))

                    # Phase A blocks interleaved with Phase B pass 1 (tg=0):
                    # the DVE stream alternates LN chains and proj copies, so
                    # PE's PSUM slots drain promptly.
                    for blk in range(4):
                        a_block(blk)
                    for i in range(8):
                        a_block(4 + i)
                        b_tg0(i)
                    for blk in range(12, NB):
                        a_block(blk)
                    for kb in range(NQ):
                        nc.sync.dma_start(out=xv[:, kb, :],
                                          in_=xb[kb * P:(kb + 1) * P, :])
                    # Phase B pass 2 (tg=1..3, pr-major; one weight load per pr)
                    for pr in range(8):
                        wk_p = wp.tile([P, NC_DCH, P], BF16, tag="wk")
                        nc.scalar.dma_start(out=wk_p, in_=wk[pr])
                        wv_p = wp.tile([P, NC_DCH, P], BF16, tag="wv")
                        nc.scalar.dma_start(out=wv_p, in_=wv[pr])
                        wq_p = wp.tile([P, NC_DCH, P], BF16, tag="wq")
                        nc.scalar.dma_start(out=wq_p, in_=wq[pr])
                        for tg in range(1, 4):
                            cols = slice(tg * 512, (tg + 1) * 512)
                            ps = mm512.tile([P, 512], F32, tag="mm")
                            for c in range(NC_DCH):
                                nc.tensor.matmul(ps, lhsT=wk_p[:, c, :],
                                                 rhs=XT[:, c, cols],
                                                 start=(c == 0),
                                                 stop=(c == NC_DCH - 1))
                            nc.vector.tensor_copy(KT[:, pr, cols], ps)
                            v_proj(tr65, pr, tg, cols, w_p=wv_p)
                            if tg == 1:
                                ps = mm512.tile([P, 512], F32, tag="mm")
                                for c in range(NC_DCH):
                                    nc.tensor.matmul(ps, lhsT=wq_p[:, c, :],
                                                     rhs=XT[:, c, cols],
                                                     start=(c == 0),
                                                     stop=(c == NC_DCH - 1))
                                nc.vector.tensor_copy(QT[:, pr, cols], ps)

                # Phase C group 0: queries 0..511 (all heads)
                with tc.tile_pool(name="otps0", bufs=2, space="PSUM") as otps0, \
                     tc.tile_pool(name="ctr0", bufs=2, space="PSUM") as ctr0:
                    for pr in range(8):
                        for hh in range(2):
                            h = pr * 2 + hh
                            hs = slice(hh * 64, (hh + 1) * 64)
                            pt = pt0p.tile([P, 2560], BF16, tag="pt0")
                            for J in range(4):
                                w = (4 - J) * P
                                for half in range(2):
                                    koff = half * 1024 + J * P
                                    off = off0(J, half)
                                    st = mm512.tile([P, 512], F32, tag="mm")
                                    nc.tensor.matmul(
                                        st[:, 0:w], lhsT=KT[hs, pr, koff:koff + P],
                                        rhs=QT[hs, pr, J * P: J * P + w],
                                        start=True, stop=True)
                                    nc.scalar.activation(
                                        out=pt[:, off: off + w],
                                        in_=st[:, 0:w], func=Exp, scale=0.125)
                                    m = mA if half == 0 else mB
                                    nc.vector.tensor_mul(pt[:, off:off + P],
                                                         pt[:, off:off + P], m)
                            for kb in range(4):
                                ot = otps0.tile([65, P], F32, tag="ot")
                                for J in range(kb + 1):
                                    for half in range(2):
                                        f = (h * 2 + half) * 8 + J
                                        o = off0(J, half) + (kb - J) * P
                                        nc.tensor.matmul(
                                            ot, lhsT=VA[:, f, :], rhs=pt[:, o:o + P],
                                            start=(J == 0 and half == 0),
                                            stop=(J == kb and half == 1))
                                epilogue(cscr0, ctr0, ot, h, kb)

            # ---- Phase C group 1 interleaved with Phase D group 0 ----
            with tc.tile_pool(name="pt1p", bufs=1) as pt1p, \
                 tc.tile_pool(name="x2tp", bufs=2) as x2tp, \
                 tc.tile_pool(name="h1p", bufs=1) as h1p, \
                 tc.tile_pool(name="w1s", bufs=2) as w1s, \
                 tc.tile_pool(name="w2s", bufs=2) as w2s, \
                 tc.tile_pool(name="lnp2", bufs=1) as lnp2, \
                 tc.tile_pool(name="scr2", bufs=1) as scr2, \
                 tc.tile_pool(name="cscr", bufs=2) as cscr, \
                 tc.tile_pool(name="mmd", bufs=3, space="PSUM") as mmd, \
                 tc.tile_pool(name="tf32p", bufs=2, space="PSUM") as tf32p, \
                 tc.tile_pool(name="tbfp", bufs=1, space="PSUM") as tbfp, \
                 tc.tile_pool(name="otps", bufs=2, space="PSUM") as otps:

                X2T = [None, None]
                h1 = [None, None]

                def d_ln2(g):
                    X2T[g] = x2tp.tile([P, NC_DCH, 512], BF16, tag="x2t", name=f"x2t{g}")
                    for s in range(4):
                        kb = g * 4 + s
                        xnb = _ln_chain(nc, lnp2, xv[:, kb, :], eps_sb, g2_t, be2_t,
                                        use_act=False)
                        _tr_to_T(nc, tbfp, xnb, X2T[g], s * P, id_bf, use_act=False)
                    h1[g] = h1p.tile([P, 32, 512], BF16, tag="h1", name=f"h1_{g}")

                def d_w1(g, f):
                    w1f = w1s.tile([P, NC_DCH, P], BF16, tag="w1f")
                    nc.scalar.dma_start(out=w1f, in_=w1t[f])
                    ps = mmd.tile([P, 512], F32, tag="mm")
                    for c in range(NC_DCH):
                        nc.tensor.matmul(ps, lhsT=w1f[:, c, :], rhs=X2T[g][:, c, :],
                                         start=(c == 0), stop=(c == NC_DCH - 1))
                    nc.scalar.activation(out=h1[g][:, f, :], in_=ps, func=Relu,
                                         bias=b1_sb[:, f:f + 1], scale=1.0)

                def d_w2(g, dd):
                    w2d = w2s.tile([P, 32, P], BF16, tag="w2d")
                    nc.sync.dma_start(out=w2d, in_=w2t[dd])
                    ps = mmd.tile([P, 512], F32, tag="mm")
                    for fc in range(32):
                        nc.tensor.matmul(ps, lhsT=w2d[:, fc, :], rhs=h1[g][:, fc, :],
                                         start=(fc == 0), stop=(fc == 31))
                    fsb = scr2.tile([P, 512], F32, tag="fsb")
                    nc.vector.tensor_scalar_add(fsb, ps, b2_sb[:, dd:dd + 1])
                    for s in range(4):
                        kb = g * 4 + s
                        tp = tf32p.tile([P, P], F32, tag="tf32")
                        nc.tensor.transpose(tp, fsb[:, s * P:(s + 1) * P], id_f32)
                        nc.vector.tensor_add(xv[:, kb, dd * P:(dd + 1) * P],
                                             xv[:, kb, dd * P:(dd + 1) * P], tp)

                def c1_unit(pr, hh):
                    h = pr * 2 + hh
                    hs = slice(hh * 64, (hh + 1) * 64)
                    pt = pt1p.tile([P, 8192], BF16, tag="pt1")
                    for J in range(8):
                        qoff = max(0, (J - 4) * P)  # first causal query in group
                        w = 512 - qoff
                        for half in range(2):
                            koff = half * 1024 + J * P
                            off = off1(J, half)
                            st = mmd.tile([P, 512], F32, tag="mm")
                            nc.tensor.matmul(
                                st[:, 0:w], lhsT=KT[hs, pr, koff:koff + P],
                                rhs=QT[hs, pr, 512 + qoff:1024], start=True, stop=True)
                            nc.scalar.activation(out=pt[:, off + qoff: off + 512],
                                                 in_=st[:, 0:w], func=Exp, scale=0.125)
                            if J >= 4:
                                m = mA if half == 0 else mB
                                do = off + qoff
                                nc.vector.tensor_mul(pt[:, do:do + P],
                                                     pt[:, do:do + P], m)
                    for kb in range(4, NQ):
                        ot = otps.tile([65, P], F32, tag="ot")
                        for J in range(kb + 1):
                            for half in range(2):
                                f = (h * 2 + half) * 8 + J
                                o = off1(J, half) + (kb - 4) * P
                                nc.tensor.matmul(
                                    ot, lhsT=VA[:, f, :], rhs=pt[:, o:o + P],
                                    start=(J == 0 and half == 0),
                                    stop=(J == kb and half == 1))
                        epilogue(cscr, tf32p, ot, h, kb)

                # interleave: 16 C1 units (front-loaded 1:2) among D-g0's
                # LN2 + 40 MM units; LN2 for g1 slots in right after the last
                # C1 epilogue so its latency hides under D-g0's PE tail.
                c_units = [(pr, hh) for pr in range(8) for hh in range(2)]
                d_units = [("w1", f) for f in range(32)] + [("w2", dd) for dd in range(8)]
                ci, di = 0, 0
                c1_unit(*c_units[ci]); ci += 1
                d_ln2(0)
                c1_unit(*c_units[ci]); ci += 1
                while ci < len(c_units):
                    for _ in range(2):
                        if di < len(d_units):
                            kind, idx = d_units[di]; di += 1
                            (d_w1 if kind == "w1" else d_w2)(0, idx)
                    c1_unit(*c_units[ci]); ci += 1
                d_ln2(1)
                while di < len(d_units):
                    kind, idx = d_units[di]; di += 1
                    (d_w1 if kind == "w1" else d_w2)(0, idx)
                # store group-0 output rows while the rest computes
                for kb in range(4):
                    nc.sync.dma_start(out=out_d[kb * P:(kb + 1) * P, :],
                                      in_=xv[:, kb, :])

                # ---- Phase D group 1 ----
                for f in range(32):
                    d_w1(1, f)
                for dd in range(8):
                    d_w2(1, dd)
                for kb in range(4, NQ):
                    nc.sync.dma_start(out=out_d[kb * P:(kb + 1) * P, :],
                                      in_=xv[:, kb, :])

    _split_drain_waits(nc)
    return nc


def _split_drain_waits(nc):
    """This walrus build gives every instruction a single hardware wait slot
    (one EVENTS struct per 64B instruction). Tile emits multi-wait
    instructions; move the excess waits onto single-wait NoOps inserted just
    before, on the same engine — identical semantics in program order."""
    for fn in nc.m.functions:
        for blk in fn.blocks:
            insts = blk.instructions
            i = 0
            while i < len(insts):
                inst = insts[i]
                si = inst.sync_info
                if si is not None and len(si.on_wait) > 1:
                    waits = list(si.on_wait)
                    inst.sync_info = mybir.SyncInfo(on_wait=[waits[-1]],
                                                    on_update=list(si.on_update))
                    for w in waits[:-1]:
                        nop = mybir.InstNoOp(name=nc.get_next_instruction_name(),
                                             ins=[], outs=[])
                        nop.engine = inst.engine
                        nop.sync_info = mybir.SyncInfo(on_wait=[w], on_update=[])
                        nc.register_instruction(nop, overwrite=True)
                        insts.insert(i, nop)
                        i += 1
                i += 1


def _prep_inputs(inputs, Wq, Wk, Wv, W1, b1, W2, b2, g1, be1, g2, be2,
                 apply_g1, apply_g2):
    bf = ml_dtypes.bfloat16
    f32 = np.float32
    inputs = np.ascontiguousarray(np.asarray(inputs, f32))
    wq_f = np.asarray(Wq, f32).transpose(1, 0, 2).reshape(D, D)
    wk_f = np.asarray(Wk, f32).transpose(1, 0, 2).reshape(D, D)
    wv_f = np.asarray(Wv, f32).transpose(1, 0, 2).reshape(D, D)

    def pair_tiles(w):  # [D, D] -> [8(pair), 128(p), 8(chunk), 128(col)]
        return np.ascontiguousarray(
            w.reshape(NC_DCH, P, 8, P).transpose(2, 1, 0, 3).astype(bf))

    wq_t, wk_t, wv_t = pair_tiles(wq_f), pair_tiles(wk_f), pair_tiles(wv_f)
    w1_t = np.ascontiguousarray(
        np.asarray(W1, f32).reshape(NC_DCH, P, 32, P).transpose(2, 1, 0, 3).astype(bf))
    w2_t = np.ascontiguousarray(
        np.asarray(W2, f32).reshape(32, P, 8, P).transpose(2, 1, 0, 3).astype(bf))
    b1_t = np.ascontiguousarray(np.asarray(b1, f32).reshape(32, P).T)
    b2_t = np.ascontiguousarray(np.asarray(b2, f32).reshape(8, P).T)

    kk, ii = np.meshgrid(np.arange(P), np.arange(P), indexing="ij")
    mask_le = (kk <= ii).astype(f32).astype(bf)   # key <= query
    mask_lt = (kk < ii).astype(f32).astype(bf)    # key <  query

    in_maps = []
    for c in range(8):
        b, o = divmod(c, 2)
        x = inputs[b]
        xb_c = np.ascontiguousarray(np.concatenate([x[o::2], x[1 - o::2]], axis=0))
        m = {"xb": xb_c, "wq": wq_t, "wk": wk_t, "wv": wv_t,
             "w1t": w1_t, "w2t": w2_t, "b1t": b1_t, "b2t": b2_t,
             "maskA": mask_le, "maskB": mask_le if o == 1 else mask_lt}
        if apply_g1:
            m["g1v"] = np.asarray(g1, f32)
            m["be1v"] = np.asarray(be1, f32)
        if apply_g2:
            m["g2v"] = np.asarray(g2, f32)
            m["be2v"] = np.asarray(be2, f32)
        in_maps.append(m)
    return in_maps


def _assemble(results):
    """results: list of 8 per-core dicts with 'out' -> full [B, T, D]."""
    out = np.empty((B, T, D), np.float32)
    for c in range(8):
        b, o = divmod(c, 2)
        out[b, o::2, :] = results[c]["out"]
    return out


def _run(inputs, Wq, Wk, Wv, W1, b1, W2, b2, g1, be1, g2, be2, **spmd_kwargs):
    apply_g1 = not (np.all(np.asarray(g1) == 1.0) and np.all(np.asarray(be1) == 0.0))
    apply_g2 = not (np.all(np.asarray(g2) == 1.0) and np.all(np.asarray(be2) == 0.0))
    nc = build_program(apply_g1, apply_g2)
    in_maps = _prep_inputs(inputs, Wq, Wk, Wv, W1, b1, W2, b2, g1, be1, g2, be2,
                           apply_g1, apply_g2)
    res = run_bass_kernel_spmd(nc, in_maps, list(range(8)), **spmd_kwargs)
    return _assemble(res.results), res


def kernel(inputs, Wq, Wk, Wv, W1, b1, W2, b2, g1, be1, g2, be2):
    out, _ = _run(inputs, Wq, Wk, Wv, W1, b1, W2, b2, g1, be1, g2, be2)
    return out
